# revision 12
# baseline (speedup 1.0000x reference)
"""JPEG compression roundtrip kernel for Trainium2 (8 NeuronCores, batch-parallel).

Self-contained: builds constants, shards batch 32 -> 8 cores x 4 images,
runs a Bass/Tile kernel per core, gathers full output.

Pipeline per image (512x512x3 f32 in [0,1)):
  S1  u8 = rne(255*x - 0.5)                  [ACT affine f32, DVE RNE -> f16]
  p1  (stat=u8 chunks, moving=color-scaled DCT f32r) -> M1 = (A@{Y,Cb,Cr})^T
  p2  (stat=DCT const f32r, moving=M1 f32r)  -> coef in PSUM
  q   deq = rne(coef*1/t)*t                  [DVE: tt mult, ts RNE 4x, tt mult 2x]
  p3  (stat=deq f16, moving=IDCT const f16/f32r) -> M3
  p4  (stat=IDCT+color consts f32r, moving=M3 f32r) -> R,G,B planes in PSUM
  S5  out = clip(v,0,255)/255 -> f16         [ACT scale-evac, DVE clamp 4x]

All matmuls use float32r moving operands (bit-exact fp32 at 1 cyc/row for
>=256-wide outputs) except p3-luma, whose const moving is f16 (1 cyc/row at
any width). The 4:2:0 chroma down/upsample is folded into the chroma DCT
matrices (E = D@P, V = 2E^T); the +-128 level shifts cancel exactly because
the DC quant step (2) divides the DC shift (1024). Output is stored as f16
(values are k/255 with k in 0..255; f16 round-off ~2^-12 relative) and
upcast to f32 on the host.
"""
import numpy as np

from concourse import bacc, bass, mybir, tile
from concourse.bass_utils import run_bass_kernel_spmd

F = np.float32
C_RNE = float(np.float32(12582912.0))  # 1.5 * 2**23
N_CORES = 8
B_PER_CORE = 4
DT = mybir.dt.float32
DTR = mybir.dt.float32r
DTH = mybir.dt.float16
QUALITY = 95

_LUMA = np.array([
    [16, 11, 10, 16, 24, 40, 51, 61],
    [12, 12, 14, 19, 26, 58, 60, 55],
    [14, 13, 16, 24, 40, 57, 69, 56],
    [14, 17, 22, 29, 51, 87, 80, 62],
    [18, 22, 37, 56, 68, 109, 103, 77],
    [24, 35, 55, 64, 81, 104, 113, 92],
    [49, 64, 78, 87, 103, 121, 120, 101],
    [72, 92, 95, 98, 112, 100, 103, 99]], dtype=F)
_CHROMA = np.array([
    [17, 18, 24, 47, 99, 99, 99, 99],
    [18, 21, 26, 66, 99, 99, 99, 99],
    [24, 26, 56, 99, 99, 99, 99, 99],
    [47, 66, 99, 99, 99, 99, 99, 99],
    [99, 99, 99, 99, 99, 99, 99, 99],
    [99, 99, 99, 99, 99, 99, 99, 99],
    [99, 99, 99, 99, 99, 99, 99, 99],
    [99, 99, 99, 99, 99, 99, 99, 99]], dtype=F)


def _qtable(base, quality):
    scale = 5000.0 / quality if quality < 50 else 200.0 - 2.0 * quality
    return np.clip(np.floor((base * scale + 50.0) / 100.0), 1.0, 255.0).astype(F)


def build_consts():
    k = np.arange(8)
    D = np.sqrt(2.0 / 8.0) * np.cos((2 * k[None, :] + 1) * k[:, None] * np.pi / 16.0)
    D[0, :] /= np.sqrt(2.0)
    D = D.astype(F)
    P = np.zeros((8, 16), F)
    for i in range(8):
        P[i, 2 * i] = 0.5
        P[i, 2 * i + 1] = 0.5
    E = (D @ P).astype(F)
    V = (2.0 * E.T).astype(F)
    QL = _qtable(_LUMA, QUALITY)
    QC = _qtable(_CHROMA, QUALITY)
    I16 = np.eye(16, dtype=F)
    I8 = np.eye(8, dtype=F)
    cY = np.array([0.299, 0.587, 0.114], F)
    cCb = np.array([-0.168736, -0.331264, 0.5], F)
    cCr = np.array([0.5, -0.418688, -0.081312], F)

    c = {}
    mv_fy = np.kron(I16, D.T).astype(F)   # [128 rowspace, 128 rowfreq]
    mv_fc = np.kron(I8, E.T).astype(F)    # [128 rowspace, 64 chroma rowfreq]
    for ch in range(3):
        c[f"mvp1_{ch}"] = np.ascontiguousarray(np.concatenate(
            [cY[ch] * mv_fy, cCb[ch] * mv_fc, cCr[ch] * mv_fc],
            axis=1).astype(np.float16))
    c["sp2y"] = mv_fy.copy()                       # [128 colspace, 128 colfreq]
    c["sp2c"] = np.ascontiguousarray(np.pad(mv_fc, ((0, 0), (0, 64))))  # padded
    c["mvp3y"] = np.kron(I16, D).astype(np.float16)   # [128 colfreq, 128 colspace]
    c["mvp3c"] = np.kron(I16, V.T).astype(np.float16)  # [128, 256]
    c["sp4y"] = np.kron(I16, D).astype(F)             # [128 rowfreq, 128 row]
    sp4c = np.kron(I16, V).T.astype(F)                # [128 rowfreq-c, 256 rows]
    wR_cr, wG_cb, wG_cr, wB_cb = 1.402, -0.344136, -0.714136, 1.772
    for h in range(2):
        sl = np.ascontiguousarray(sp4c[:, 128 * h:128 * (h + 1)])
        c[f"sp4c_h{h}_rcr"] = (F(wR_cr) * sl).astype(F)
        c[f"sp4c_h{h}_gcb"] = (F(wG_cb) * sl).astype(F)
        c[f"sp4c_h{h}_gcr"] = (F(wG_cr) * sl).astype(F)
        c[f"sp4c_h{h}_bcb"] = (F(wB_cb) * sl).astype(F)
    # quant tables: [partition 128, free 128]; value = Q[rowfreq=ff%8, colfreq=pp%8]
    pp, ff = np.meshgrid(np.arange(128), np.arange(128), indexing="ij")
    tY = QL[ff % 8, pp % 8].astype(F)
    tC = QC[ff % 8, pp % 8].astype(F)
    c["rtaby"] = (1.0 / tY).astype(F)
    c["taby"] = tY.astype(np.float16)
    c["rtabc"] = (1.0 / tC).astype(F)
    c["tabc"] = tC.astype(np.float16)
    return c


CONST_SHAPES = {
    "mvp1_0": ((128, 256), DTH), "mvp1_1": ((128, 256), DTH), "mvp1_2": ((128, 256), DTH),
    "sp2y": ((128, 128), DTR), "sp2c": ((128, 128), DTR),
    "mvp3y": ((128, 128), DTH), "mvp3c": ((128, 256), DTH),
    "sp4y": ((128, 128), DTR),
    "sp4c_h0_rcr": ((128, 128), DTR), "sp4c_h0_gcb": ((128, 128), DTR),
    "sp4c_h0_gcr": ((128, 128), DTR), "sp4c_h0_bcb": ((128, 128), DTR),
    "sp4c_h1_rcr": ((128, 128), DTR), "sp4c_h1_gcb": ((128, 128), DTR),
    "sp4c_h1_gcr": ((128, 128), DTR), "sp4c_h1_bcb": ((128, 128), DTR),
    "rtaby": ((128, 128), DT), "taby": ((128, 128), DTH),
    "rtabc": ((128, 128), DT), "tabc": ((128, 128), DTH),
}


def build_nc():
    Alu = mybir.AluOpType
    Act = mybir.ActivationFunctionType
    nc = bacc.Bacc("TRN2", target_bir_lowering=False, debug=False,
                   num_devices=N_CORES)
    x_d = nc.dram_tensor("x", [B_PER_CORE, 512, 512, 3], DT,
                         kind="ExternalInput").ap()
    o_d = nc.dram_tensor("out", [B_PER_CORE, 512, 512, 3], DTH,
                         kind="ExternalOutput").ap()
    cd = {k: nc.dram_tensor(k, list(s), d, kind="ExternalInput").ap()
          for k, (s, d) in CONST_SHAPES.items()}

    with tile.TileContext(nc) as tc:
        with (
            tc.tile_pool(name="cpool", bufs=1) as cpool,
            tc.tile_pool(name="iopool", bufs=3) as iopool,
            tc.tile_pool(name="u8pool", bufs=5) as u8pool,
            tc.tile_pool(name="m1pool", bufs=5) as m1pool,
            tc.tile_pool(name="qypool", bufs=5) as qypool,
            tc.tile_pool(name="qcpool", bufs=5) as qcpool,
            tc.tile_pool(name="m3ypool", bufs=5) as m3ypool,
            tc.tile_pool(name="m3cpool", bufs=3) as m3cpool,
            tc.tile_pool(name="otpool", bufs=3) as otpool,
            tc.tile_pool(name="pspool", bufs=3, space="PSUM") as pspool,
            tc.tile_pool(name="psmid", bufs=2, space="PSUM") as psmid,
            tc.tile_pool(name="psp4", bufs=3, space="PSUM") as psp4,
        ):
            ct = {}
            for k, (s, d) in CONST_SHAPES.items():
                ct[k] = cpool.tile(list(s), d, tag=k, name=k)
                nc.sync.dma_start(out=ct[k][:], in_=cd[k][:])
            rtaby4 = ct["rtaby"][:].unsqueeze(1).broadcast_to([128, 4, 128])
            taby4 = ct["taby"][:].unsqueeze(1).broadcast_to([128, 4, 128])
            rtabc2 = ct["rtabc"][:].unsqueeze(1).broadcast_to([128, 2, 128])
            tabc2 = ct["tabc"][:].unsqueeze(1).broadcast_to([128, 2, 128])

            for b in range(B_PER_CORE):
                # ---- S1: load + floor(255*x) = rne(255*x - 0.5) ----
                u8 = []
                for r in range(4):
                    xin = iopool.tile([128, 512, 3], DT, tag="xin", name="xin")
                    nc.sync.dma_start(out=xin[:], in_=x_d[b, 128 * r:128 * (r + 1)])
                    u8t = u8pool.tile([128, 512, 3], DTH, tag="u8", name="u8t")
                    nc.scalar.activation(u8t[:], xin[:], Act.Copy,
                                         bias=-0.5, scale=255.0)
                    u8.append(u8t)

                # ---- p1: M1 = (A @ {Y,Cb,Cr})^T, color fused via PSUM accum ----
                m1 = []
                for jc in range(4):
                    psA = pspool.tile([128, 2, 256], DT, tag="ps1", name="psA")
                    psB = pspool.tile([128, 2, 256], DT, tag="ps1", name="psB")
                    for r in range(4):
                        pst = psA if r < 2 else psB
                        g = r % 2
                        for ch in range(3):
                            nc.tensor.matmul(
                                pst[:, g, :],
                                u8[r][:, 128 * jc:128 * (jc + 1), ch],
                                ct[f"mvp1_{ch}"][:],
                                start=(ch == 0), stop=(ch == 2))
                    m1t = m1pool.tile([128, 4, 256], DTR, tag="m1", name="m1t")
                    nc.vector.tensor_copy(m1t[:, 0:2, :], psA[:])
                    nc.vector.tensor_copy(m1t[:, 2:4, :], psB[:])
                    m1.append(m1t)

                # ---- p2 + quant: luma ----
                qy = []
                for jc in range(4):
                    ps2 = psmid.tile([128, 4, 128], DT, tag="psm", name="ps2")
                    nc.tensor.matmul(ps2[:], ct["sp2y"][:],
                                     m1[jc][:, :, 0:128], start=True, stop=True)
                    qt = qypool.tile([128, 4, 128], DTH, tag="qy", name="qty")
                    nc.vector.tensor_tensor(
                        out=qt[:], in0=ps2[:], in1=rtaby4, op=Alu.mult)
                    nc.vector.tensor_scalar(
                        out=qt[:], in0=qt[:], scalar1=C_RNE, scalar2=C_RNE,
                        op0=Alu.add, op1=Alu.subtract)
                    nc.gpsimd.tensor_tensor(
                        out=qt[:], in0=qt[:], in1=taby4, op=Alu.mult)
                    qy.append(qt)

                # ---- p2 + quant: chroma (two jc halves share one PSUM tile) ----
                qc = {0: [], 1: []}
                for chi in (0, 1):
                    for t_ in range(2):
                        qt = qcpool.tile([128, 2, 128], DTH, tag="qc", name="qtc")
                        for half in range(2):
                            jc = 2 * t_ + half
                            lo = 128 + 64 * chi
                            psc = psmid.tile([128, 2, 128], DT, tag="psm",
                                             name="psc")
                            nc.tensor.matmul(
                                psc[:], ct["sp2c"][:],
                                m1[jc][:, :, lo:lo + 64],
                                start=True, stop=True)
                            nc.vector.tensor_tensor(
                                out=qt[64 * half:64 * (half + 1), :, :],
                                in0=psc[0:64, :, :], in1=rtabc2[0:64, :, :],
                                op=Alu.mult)
                        nc.vector.tensor_scalar(
                            out=qt[:], in0=qt[:], scalar1=C_RNE, scalar2=C_RNE,
                            op0=Alu.add, op1=Alu.subtract)
                        nc.gpsimd.tensor_tensor(
                            out=qt[:], in0=qt[:], in1=tabc2, op=Alu.mult)
                        qc[chi].append(qt)

                # ---- p3: luma col-IDCT -> M3y [rowfreq, colspace] ----
                m3y = []
                for rchunk in range(4):
                    ps3 = psmid.tile([128, 4, 128], DT, tag="psm", name="ps3")
                    for jc in range(4):
                        nc.tensor.matmul(
                            ps3[:, jc, :], qy[jc][:, rchunk, :],
                            ct["mvp3y"][:], start=True, stop=True)
                    mt = m3ypool.tile([128, 4, 128], DTR, tag="m3y", name="mty")
                    nc.scalar.copy(mt[:], ps3[:])
                    m3y.append(mt)

                # ---- p3: chroma col-IDCT + h-upsample -> M3c ----
                m3c = {0: [], 1: []}
                for chi in (0, 1):
                    for rc in range(2):
                        ps3 = psmid.tile([128, 2, 256], DT, tag="psm", name="ps3c")
                        for t_ in range(2):
                            nc.tensor.matmul(
                                ps3[:, t_, :], qc[chi][t_][:, rc, :],
                                ct["mvp3c"][:], start=True, stop=True)
                        mt = m3cpool.tile([128, 2, 256], DTR, tag="m3c", name="mtc")
                        nc.vector.tensor_copy(mt[:], ps3[:])
                        m3c[chi].append(mt)

                # ---- p4 + color + clamp + store ----
                for r in range(4):
                    rc, half = divmod(r, 2)
                    psR = psp4.tile([128, 512], DT, tag="ps4", name="psR")
                    psG = psp4.tile([128, 512], DT, tag="ps4", name="psG")
                    psB4 = psp4.tile([128, 512], DT, tag="ps4", name="psB4")
                    my = m3y[r][:]
                    mcb = m3c[0][rc][:]
                    mcr = m3c[1][rc][:]

                    def _acc(ps, terms):
                        n = len(terms)
                        for i, (cname, mv) in enumerate(terms):
                            nc.tensor.matmul(ps[:], ct[cname][:], mv,
                                             start=(i == 0), stop=(i == n - 1))
                    _acc(psR, [("sp4y", my), (f"sp4c_h{half}_rcr", mcr)])
                    _acc(psG, [("sp4y", my), (f"sp4c_h{half}_gcb", mcb),
                               (f"sp4c_h{half}_gcr", mcr)])
                    _acc(psB4, [("sp4y", my), (f"sp4c_h{half}_bcb", mcb)])
                    ot = otpool.tile([128, 512, 3], DTH, tag="ot", name="ot")
                    inv255 = float(F(1.0) / F(255.0))
                    for chn, ps in ((0, psR), (1, psG), (2, psB4)):
                        nc.scalar.activation(ot[:, :, chn], ps[:], Act.Copy,
                                             bias=0.0, scale=inv255)
                    nc.vector.tensor_scalar(
                        out=ot[:], in0=ot[:], scalar1=1.0, scalar2=0.0,
                        op0=Alu.min, op1=Alu.max)
                    nc.sync.dma_start(out=o_d[b, 128 * r:128 * (r + 1)], in_=ot[:])

    nc.compile()
    return nc


_CACHE = {}


def kernel(x: np.ndarray) -> np.ndarray:
    assert x.shape == (32, 512, 512, 3)
    if "nc" not in _CACHE:
        _CACHE["nc"] = build_nc()
        _CACHE["consts"] = build_consts()
    nc = _CACHE["nc"]
    consts = _CACHE["consts"]
    xs = np.ascontiguousarray(x.astype(F))
    in_maps = []
    for i in range(N_CORES):
        m = {"x": xs[B_PER_CORE * i:B_PER_CORE * (i + 1)]}
        m.update(consts)
        in_maps.append(m)
    res = run_bass_kernel_spmd(nc, in_maps, list(range(N_CORES)))
    out = np.concatenate([res.results[i]["out"] for i in range(N_CORES)], axis=0)
    return out.astype(np.float32)
